# revision 70
# baseline (speedup 1.0000x reference)
"""Trainium2 Bass kernel: attention layer with KV cache, tensor-parallel over heads.

Sharding (8 NeuronCores): Megatron-style TP over the 32 heads -> 4 heads/core.
  - wq/wk/wv: column-parallel (each core owns a [512, 4096] output shard)
  - wo: row-parallel (each core owns wo[:, c*512:(c+1)*512]); cores emit
    partial o-proj outputs which the host sums (RowParallel unshard).
  - cache_k/cache_v: sharded along the head axis; history rows/positions are
    gathered host-side from batch_exec/start_pos (pure indexing).

Device layout trick: Q/K are projected directly in transposed [head_dim, tok]
layout (head dim = PSUM partitions), V in natural [tok, head_dim] layout, so
scores^T, PV, and the o-projection all consume each other's outputs as
matmul operands with zero on-device transposes.  Softmax runs without the
max-subtraction (scores are O(1), exp is safe in f32) so the kv-axis
(partition-axis) row-sum comes from a ones-vector matmul; normalization is a
rank-1 broadcast matmul of 1/r.  exp runs IN-PLACE in PSUM: the ACT engine's
fast port is PSUM on both sides (the ACT->SBUF write path measured ~8x
slower per element).

New-token compaction: router[:, :, 0] gates ~half the new cache positions
off for every query, so their K/V projections and score/PV columns are dead
work.  The host compacts each batch's unmasked new tokens (order preserved)
and the program projects/attends over pad16(count_b) columns per batch,
with exact causal narrowing from the compacted token positions (chunk at
compact offset j0 only reaches queries s >= orig_pos(j0)).  Batches with
>384 usable tokens fall back to uncompacted identity order (SBUF would not
fit the staging buffer, and compaction stops paying).  The program is built
per-(counts, positions) and cached, so repeated calls with the same router
compile once.

Scheduling: the PE queue is strict FIFO, so emission order is execution
order.  Attention is software-pipelined per head (rowsum/PV matmuls lag two
chunks behind the score matmuls), and the PREVIOUS batch's o-projection
groups are interleaved into the attention beats as dense PE filler, so the
PE never head-of-line blocks on the exp->gate chain.
"""

import numpy as np
import ml_dtypes

import concourse.bass as bass
import concourse.bacc as bacc
import concourse.tile as tile
from concourse import mybir
from concourse.bass_utils import run_bass_kernel_spmd

BF16 = np.dtype(ml_dtypes.bfloat16)

# Problem shape (hardcoded per the task contract)
BSZ = 8
SEQ = 512
DIM = 4096
NH = 32
HD = 128
START = 512
KV = START + SEQ          # 1024
NC = 8                    # cores
HPC = NH // NC            # 4 heads per core
HF = HPC * HD             # 512 local features
P = 128
KC = DIM // P             # 32 contraction chunks
SC = SEQ // P             # 4 seq chunks (also history kv chunks)
ROPE_BASE = 10000.0

FP32 = mybir.dt.float32
BF16D = mybir.dt.bfloat16


def build_program(nkvs, newpos, wu_bufs=4, hist_ones=True, po_bufs=3,
                  tails=()):
    # tails: tuple of (b, slot, tl) — batches whose last V chunk (tl<=32
    # tokens) is projected in batch 0's phase A as one shared 128-partition
    # chunk: each tail's matmuls go to output partitions [32*slot, 32*slot+tl)
    # via col-group tile_position, so the 4 tails' streams run concurrently
    # and 4 nearly-empty chunk passes collapse into one.
    # Every DRAM parameter is pre-packed host-side into the exact SBUF tile
    # layout (partition-major), so each DMA is a contiguous >=8KB-per-partition
    # stream: ~128 fat descriptors instead of thousands of 256B ones.
    # nkvs: per-batch padded compacted new-token counts (multiples of 16).
    # newpos[b][j] = original position of compacted token j (for exact causal
    # narrowing): new chunk starting at compact j0 only reaches queries
    # s >= newpos[b][j0].
    # hist_ones: host verified the history gate (causal AND cache_mask) is
    # all-ones, so history chunks skip the gate multiply (plain copy) and the
    # gate tensor only carries the new-token chunks.
    nkvm = max(nkvs)
    tcnm = (nkvm + P - 1) // P        # max new-token kv chunks
    nchm = SC + tcnm                  # max total kv chunks per head
    # max per-batch OWN v chunks (tail-managed batches keep only full chunks)
    _tbs = {b for b, _, _ in tails}
    vchm = max((nkvs[b] // P) if b in _tbs else (nkvs[b] + P - 1) // P
               for b in range(BSZ))
    gchm = tcnm if hist_ones else nchm  # gate chunks stored
    # batches with nkv == SEQ run uncompacted straight from the resident x
    # tile; the compacted-x staging buffer only needs the compacted batches
    nkvc = max([n for n in nkvs if n < SEQ], default=16)

    tail_of = {b: (slot, tl) for b, slot, tl in tails}
    ntailc = sum(tl for _, _, tl in tails)

    nc = bacc.Bacc(None, target_bir_lowering=False)
    x_d = nc.declare_dram_parameter("xp", [BSZ, P, KC, SEQ], BF16D, isOutput=False)
    xkv_d = nc.declare_dram_parameter("xkvp", [BSZ, P, KC, nkvc], BF16D, isOutput=False)
    if tails:
        xtail_d = nc.declare_dram_parameter("xtailp", [P, KC, ntailc], BF16D, isOutput=False)
    wqk_d = nc.declare_dram_parameter("wqk", [2 * HPC, P, KC, P], BF16D, isOutput=False)
    wv_d = nc.declare_dram_parameter("wvp", [P, KC, HF], BF16D, isOutput=False)
    woT_d = nc.declare_dram_parameter("wop", [P, HPC, DIM], BF16D, isOutput=False)
    rope_d = nc.declare_dram_parameter("ropep", [BSZ, P, 2, SEQ + nkvm], FP32, isOutput=False)
    att_d = nc.declare_dram_parameter("attp", [BSZ, P, 8 + gchm, SEQ], BF16D, isOutput=False)
    out_d = nc.declare_dram_parameter("out", [BSZ * SEQ, DIM], BF16D, isOutput=True)

    from contextlib import ExitStack

    with ExitStack() as ctx:
        tc = ctx.enter_context(tile.TileContext(nc))
        cpool = ctx.enter_context(tc.tile_pool(name="const", bufs=1))
        # 6 bufs = 3 weight units in flight: the K-proj units are consumed in
        # ~3.7us each (272-col matmuls) but take ~4.5us to supply end-to-end,
        # so 2-deep prefetch starves once per batch (and the resulting >3.4us
        # PE idle re-throttles HAM to 1.2GHz for the next ~3.4us).
        wupool = ctx.enter_context(tc.tile_pool(name="wu", bufs=wu_bufs))
        xpool = ctx.enter_context(tc.tile_pool(name="xb", bufs=1))
        rpool = ctx.enter_context(tc.tile_pool(name="rope", bufs=1))
        qkvpool = ctx.enter_context(tc.tile_pool(name="qkv", bufs=2))
        hpool = ctx.enter_context(tc.tile_pool(name="hist", bufs=1))
        # bufs=1: the RoPE mul/mul/mul/add chain is DVE-serial regardless,
        # so double-buffering t1/t2 buys no overlap
        wkpool = ctx.enter_context(tc.tile_pool(name="work", bufs=1))
        epool = ctx.enter_context(tc.tile_pool(name="ee", bufs=2))
        apool = ctx.enter_context(tc.tile_pool(name="at", bufs=2))
        smpool = ctx.enter_context(tc.tile_pool(name="small", bufs=1))
        popool = ctx.enter_context(tc.tile_pool(name="po", bufs=po_bufs))
        pA = ctx.enter_context(tc.tile_pool(name="pA", bufs=2, space="PSUM"))
        pS = ctx.enter_context(tc.tile_pool(name="pS", bufs=2, space="PSUM"))
        pR = ctx.enter_context(tc.tile_pool(name="pR", bufs=1, space="PSUM"))
        pOT = ctx.enter_context(tc.tile_pool(name="pOT", bufs=1, space="PSUM"))
        pP = ctx.enter_context(tc.tile_pool(name="pP", bufs=2, space="PSUM"))
        if True:
            # ---- constants (weights emitted after b0's hot DMAs, see below) ----
            ones_bf = cpool.tile([P, 1], BF16D)
            nc.gpsimd.memset(ones_bf[:], 1.0)
            wv_s = cpool.tile([P, KC, HF], BF16D)
            woT_s = cpool.tile([P, HPC, DIM], BF16D)
            if tails:
                vtailb = cpool.tile([P, HF], BF16D)
                xtailb = cpool.tile([P, KC, ntailc], BF16D)

            def oproj_unit(aT_prev, ts_prev, k, out_eng=None):
                # one (seq-chunk, out-chunk-pair) group of the previous
                # batch's o-projection: dense PE filler between the
                # exp/gate-gated attention beats.
                sc4, og = divmod(k, 4)
                pout = popool.tile([P, 2, HF], BF16D, tag="pout")
                for oi in range(2):
                    oc = og * 2 + oi
                    pp = pP.tile([P, HF], FP32, tag="pp")
                    for h in range(HPC):
                        nc.tensor.matmul(
                            pp[:],
                            aT_prev[:, h, sc4 * P:(sc4 + 1) * P],
                            woT_s[:, h, oc * HF:(oc + 1) * HF],
                            start=(h == 0), stop=(h == HPC - 1),
                        ).annotate("op")
                    nc.vector.tensor_copy(pout[:, oi, :], pp[:]).annotate("opcp")
                (out_eng or nc.gpsimd).dma_start(
                    out_d[ts_prev + sc4 * P: ts_prev + (sc4 + 1) * P,
                          og * 2 * HF:(og * 2 + 2) * HF],
                    pout[:],
                ).annotate("d_out")

            prev_o = None  # (aT, ts) of the batch whose o-proj is pending
            for b in range(BSZ):
                ts = b * SEQ
                nkv = nkvs[b]
                tcn = (nkv + P - 1) // P
                nch = SC + tcn
                vch = (nkv + P - 1) // P
                # b0 runs its phase A as K -> V -> Q: nothing is prefetched
                # yet, and the 4MB x stream alone takes ~11us vs Q-proj's
                # ~7us/head demand. Projecting K (from the small compacted
                # xkv on the scalar queue) and V first lets x land in the
                # background so Q runs dense.
                kfirst = b == 0 and nkv < SEQ
                ufirst = HPC if kfirst else 0
                wuf0 = wupool.tile([P, 16, P], BF16D, tag="wu")
                wuf1 = wupool.tile([P, 16, P], BF16D, tag="wu")
                wu_first = [wuf0, wuf1]
                # at program start the scalar DGE queue is empty: split the
                # first weight unit across both queues so the first matmul
                # group starts ~0.25MB+1MB into the sync stream (b>0 keeps
                # everything on sync -- the scalar queue carries the rope/
                # history/gate tensors there and must not be delayed)
                eng1 = nc.scalar if b == 0 else nc.sync
                for wc in range(4):
                    eng = (nc.sync, nc.sync, eng1, eng1)[wc]
                    eng.dma_start(wu_first[wc // 2][:, (wc % 2) * 8:(wc % 2 + 1) * 8, :],
                                  wqk_d[ufirst, :, wc * 8:(wc + 1) * 8, :]).annotate("d_wu0")
                xb = xpool.tile([P, KC, SEQ], BF16D)
                xkvb = xpool.tile([P, KC, nkvc], BF16D, tag="xkv")

                def emit_x():
                    xsplits = (0, 2, 4, 8, 16, 24, 32) if b == 0 else (0, 8, 16, 24, 32)
                    for xa, xbnd in zip(xsplits, xsplits[1:]):
                        nc.sync.dma_start(
                            xb[:, xa:xbnd, :],
                            x_d[b, :, xa:xbnd, :],
                        ).annotate("d_x")

                def emit_v():
                    srckv = xb if nkv == SEQ else xkvb
                    vch_own = (nkv // P) if b in tail_of else vch
                    for vc in range(vch_own):  # v, natural layout
                        pc = min(P, nkv - vc * P)
                        ps = pA.tile([P, SEQ], FP32, tag="pa")
                        for kc in range(KC):
                            nc.tensor.matmul(
                                ps[0:pc, :HF], srckv[:, kc, vc * P:vc * P + pc], wv_s[:, kc, :],
                                start=(kc == 0), stop=(kc == KC - 1),
                            ).annotate("v")
                        nc.vector.tensor_copy(v_b[0:pc, vc, :], ps[0:pc, :HF]).annotate("vcp")

                if kfirst:
                    # xkv leads the scalar queue: K-proj consumes it ~10us in
                    for xc in range(2):
                        nc.scalar.dma_start(
                            xkvb[:, xc * 16:(xc + 1) * 16, :nkv],
                            xkv_d[b, :, xc * 16:(xc + 1) * 16, :nkv],
                        ).annotate("d_xkv")
                    emit_x()
                else:
                    emit_x()
                # small hot tensors ride the scalar engine's DGE queues so
                # they are not stuck behind the multi-MB sync-queue streams.
                # They are packed host-side into TWO tensors (rope fp32, rest
                # bf16): each dma_start issue occupies the ACT sequencer for
                # ~3.5us, and the ACT queue also runs the attention exps.
                rope_b = rpool.tile([P, 2, SEQ + nkvm], FP32, tag="rope")
                nc.scalar.dma_start(rope_b[:, :, :SEQ + nkv], rope_d[b, :, :, :SEQ + nkv]).annotate("d_rope")
                gch = tcn if hist_ones else nch
                # attb rows: 0-3 history kT (per head), 4-7 history v (per
                # head, 4x128 tokens), 8+ new-token gate chunks
                attb = hpool.tile([P, 8 + gchm, SEQ], BF16D)
                nc.scalar.dma_start(attb[:, :8 + gch, :], att_d[b, :, :8 + gch, :]).annotate("d_att")

                # ---- phase A: QKV projections (+RoPE for q/k) ----
                qT_b = qkvpool.tile([P, HPC, SEQ], BF16D, tag="qT")
                kT_b = qkvpool.tile([P, HPC, nkvm], BF16D, tag="kT")
                v_b = qkvpool.tile([P, vchm, HF], BF16D, tag="v", bufs=1)

                proj_order = (1, 0) if kfirst else (0, 1)
                for proj in proj_order:  # 0=q (full tokens), 1=k (compacted)
                    if proj == 1 and nkv < SEQ and not kfirst:
                        # xkv is only needed by the k/v units; emitting its
                        # DMA here keeps the q weight-units at the head of
                        # the sync queue (NOTE: the scalar DGE queue is
                        # bandwidth-limited -- bulk streams belong on sync).
                        for xc in range(2):
                            nc.sync.dma_start(
                                xkvb[:, xc * 16:(xc + 1) * 16, :nkv],
                                xkv_d[b, :, xc * 16:(xc + 1) * 16, :nkv],
                            ).annotate("d_xkv")
                    if b == 0 and proj == 1:
                        # wv rides the scalar queue in kfirst mode (the sync
                        # queue carries k-units + x + q-units back-to-back)
                        wv_eng = nc.scalar if kfirst else nc.sync
                        wv_eng.dma_start(wv_s[:, 0:16, :], wv_d[:, 0:16, :]).annotate("d_wv")
                        wv_eng.dma_start(wv_s[:, 16:32, :], wv_d[:, 16:32, :]).annotate("d_wv")
                        if tails:
                            nc.sync.dma_start(xtailb[:], xtail_d[:]).annotate("d_xtail")
                    if kfirst and proj == 0:
                        emit_x()
                    if proj == 0:
                        dst, src, roff, ncols = qT_b, xb, 0, SEQ
                    else:
                        src = xb if nkv == SEQ else xkvb
                        dst, roff, ncols = kT_b, SEQ, nkv
                    for h in range(HPC):
                        if proj == proj_order[0] and h == 0:
                            wu = wu_first
                        else:
                            wu0 = wupool.tile([P, 16, P], BF16D, tag="wu")
                            wu1 = wupool.tile([P, 16, P], BF16D, tag="wu")
                            wu = [wu0, wu1]
                            u = proj * HPC + h
                            # one fat DMA per tile: the 584ns-per-instruction
                            # sync-queue issue cost is what starves the fast
                            # K-proj units, not the transfer itself.
                            # In kfirst mode the k units ride the scalar
                            # queue: their pool-recycling hazard waits would
                            # otherwise head-of-line block the x stream on
                            # the sync queue.
                            ueng = nc.scalar if (kfirst and proj == 1) else nc.sync
                            for half in range(2):
                                ueng.dma_start(wu[half][:], wqk_d[u, :, half * 16:half * 16 + 16, :]).annotate("d_wu")
                        ps = pA.tile([P, SEQ], FP32, tag="pa")
                        for kc in range(KC):
                            nc.tensor.matmul(
                                ps[:, :ncols], wu[kc // 16][:, kc % 16, :], src[:, kc, :ncols],
                                start=(kc == 0), stop=(kc == KC - 1),
                            ).annotate("qk")
                        # RoPE: dst = ps*cos + shift64(ps)*sin_signed
                        t1 = wkpool.tile([P, SEQ], FP32, tag="t1")
                        nc.vector.tensor_mul(t1[:, :ncols], ps[:, :ncols], rope_b[:, 0, roff:roff + ncols]).annotate("rope")
                        t2 = wkpool.tile([P, SEQ], FP32, tag="t2")
                        H2 = HD // 2
                        nc.vector.tensor_mul(t2[0:H2, :ncols], ps[H2:P, :ncols], rope_b[0:H2, 1, roff:roff + ncols]).annotate("rope")
                        nc.vector.tensor_mul(t2[H2:P, :ncols], ps[0:H2, :ncols], rope_b[H2:P, 1, roff:roff + ncols]).annotate("rope")
                        nc.vector.tensor_add(dst[:, h, :ncols], t1[:, :ncols], t2[:, :ncols]).annotate("rope")
                    if kfirst and proj == 1:
                        emit_v()

                if not kfirst:
                    emit_v()

                if b == 0 and tails:
                    # shared tail chunk: project every tail batch's leftover
                    # tokens in one pass; the slots' matmuls go to disjoint
                    # 32-aligned output partition groups (distinct col-groups
                    # -> concurrent streams). Each slot accumulates in its OWN
                    # psum bank: interleaved accumulation groups sharing one
                    # bank corrupt each other through the per-bank has_written
                    # clear (measured: slots 1-3 garbage). The attention-score
                    # pool pS is idle during phase A, so its banks serve as
                    # the extra two.
                    psts = []
                    for ti in range(len(tails)):
                        pst = (pA if ti < 2 else pS).tile([P, SEQ], FP32,
                                                          tag=("pa" if ti < 2 else "ps"))
                        psts.append(pst)
                    for kc in range(KC):
                        off = 0
                        for ti, (tb, slot, tl) in enumerate(tails):
                            nc.tensor.matmul(
                                psts[ti][32 * slot:32 * slot + tl, :HF],
                                xtailb[:, kc, off:off + tl], wv_s[:, kc, :],
                                start=(kc == 0), stop=(kc == KC - 1),
                                skip_group_check=True,
                                tile_position=(0, 32 * slot),
                            ).annotate("v")
                            off += tl
                    for ti, (tb, slot, tl) in enumerate(tails):
                        nc.vector.tensor_copy(
                            vtailb[32 * slot:32 * slot + tl, :],
                            psts[ti][32 * slot:32 * slot + tl, :HF]).annotate("vcp")

                if b == 0:
                    # woT is first needed by b0's o-projection, which runs as
                    # filler inside b1's attention (~120us in). Emitting it
                    # here keeps its 4MB out of b0's DMA-deficit window
                    # (b0's phase A already pulls ~12MB against a ~45us span).
                    nc.scalar.dma_start(woT_s[:, 0:2, :], woT_d[:, 0:2, :]).annotate("d_wo")
                    nc.scalar.dma_start(woT_s[:, 2:4, :], woT_d[:, 2:4, :]).annotate("d_wo")

                # ---- phase B: attention, software-pipelined per head ----
                # Chunk list: 4 history chunks (s0=0) then compacted new
                # chunks; new chunk at compact offset j0 only reaches queries
                # s >= j0 (orig position >= compact index), so narrow ops.
                # The PE queue is strict FIFO, so the emission order below IS
                # the execution order: rowsum/PV for chunk ci-2 are emitted
                # between score matmuls so the PE never heads-of-line blocks
                # on the exp->gate chain, and o-proj groups of the PREVIOUS
                # batch are sprinkled in as dense filler.
                aT = apool.tile([P, HPC, SEQ], BF16D)
                ounits = list(range(16)) if prev_o is not None else []
                bslot = tail_of.get(b, (0, 0))[0] * 32 if b in tail_of else 0
                for h in range(HPC):
                    # chunk entries: (k lhsT, v lhsT, pc, s0, p0) where p0 is
                    # the partition base the chunk's scores/ee live at (only
                    # nonzero for the shared-tail chunk)
                    chunks = []
                    for t in range(SC):
                        chunks.append((attb[:, h, t * P:(t + 1) * P],
                                       attb[:, HPC + h, t * P:(t + 1) * P], P, 0, 0))
                    for tcn_i in range(tcn):
                        j0 = tcn_i * P
                        pc = min(P, nkv - j0)
                        s0 = int(newpos[b][j0]) if j0 < len(newpos[b]) else SEQ - 1
                        # round the causal-narrowing start down to 8 elements:
                        # odd s0 makes every narrowed PSUM/SBUF access pattern
                        # straddle the 8-byte PSUM cachelines; the gate is
                        # genuinely zero on [s0&~7, s0) so the result is
                        # unchanged
                        s0 &= ~7
                        if b in tail_of and tcn_i == tcn - 1:
                            chunks.append((kT_b[:, h, j0:j0 + pc],
                                           vtailb[bslot:bslot + pc, h * HD:(h + 1) * HD],
                                           pc, s0, bslot))
                        else:
                            chunks.append((kT_b[:, h, j0:j0 + pc],
                                           v_b[0:pc, tcn_i, h * P:(h + 1) * P], pc, s0, 0))
                    # ee is a 4-slot ring over kv chunks: rs/pv consume chunk
                    # ci two chunks behind the score matmuls, so slot ci%4 is
                    # free again by the time chunk ci+4 writes it
                    ee = epool.tile([P, 4, SEQ], BF16D)
                    pr = pR.tile([1, SEQ], FP32, tag="pr")
                    po = pOT.tile([P, SEQ], FP32, tag="po")

                    def rs_pv(ci):
                        _, vlhs, pc, s0, p0 = chunks[ci]
                        nc.tensor.matmul(
                            pr[:, s0:], ones_bf[p0:p0 + pc, :], ee[p0:p0 + pc, ci % 4, s0:],
                            start=(ci == 0), stop=(ci == nch - 1),
                            skip_group_check=True,
                            tile_position=(p0, 0),
                        ).annotate("rs")
                        nc.tensor.matmul(
                            po[:, s0:], vlhs, ee[p0:p0 + pc, ci % 4, s0:],
                            start=(ci == 0), stop=(ci == nch - 1),
                            skip_group_check=True,
                            tile_position=(p0, 0),
                        ).annotate("pv")

                    for ci, (klhs, vlhs, pc, s0, p0) in enumerate(chunks):
                        pscr = pS.tile([P, SEQ], FP32, tag="ps")
                        nc.tensor.matmul(pscr[p0:p0 + pc, s0:], klhs, qT_b[:, h, s0:],
                                         start=True, stop=True,
                                         tile_position=(0, p0)).annotate("sc")
                        # exp in-place in PSUM: ScalarE's fast port is PSUM on
                        # both sides; ACT->SBUF measured ~8x slower.
                        nc.scalar.activation(pscr[p0:p0 + pc, s0:], pscr[p0:p0 + pc, s0:], mybir.ActivationFunctionType.Exp).annotate("exp")
                        if hist_ones and ci < SC:
                            # history gate is provably all-ones: plain copy
                            nc.vector.tensor_copy(ee[0:pc, ci % 4, s0:], pscr[0:pc, s0:]).annotate("gate")
                        else:
                            gci = ci - SC if hist_ones else ci
                            nc.vector.tensor_mul(ee[p0:p0 + pc, ci % 4, s0:], pscr[p0:p0 + pc, s0:], attb[0:pc, 8 + gci, s0:]).annotate("gate")
                        if ci >= 2:
                            rs_pv(ci - 2)
                            if ci % 2 == 0 and ounits:
                                oproj_unit(prev_o[0], prev_o[1], ounits.pop(0))
                    rs_pv(nch - 2)
                    if ounits:
                        oproj_unit(prev_o[0], prev_o[1], ounits.pop(0))
                    rs_pv(nch - 1)
                    rinv = smpool.tile([1, SEQ], FP32, tag="rinv")
                    nc.vector.reciprocal_approx_fast(rinv[:], pr[:])
                    rb_s = smpool.tile([P, SEQ], FP32, tag="rbs")
                    nc.gpsimd.partition_broadcast(rb_s[:], rinv[:]).annotate("rbc")
                    nc.vector.tensor_mul(aT[:, h, :], po[:], rb_s[:]).annotate("norm")
                # drain any o-proj groups this batch's slots didn't absorb
                while ounits:
                    oproj_unit(prev_o[0], prev_o[1], ounits.pop(0))
                prev_o = (aT, ts)

            # final batch's o-projection (no next attention phase to hide in)
            for k in range(16):
                oproj_unit(prev_o[0], prev_o[1], k)
    nc.finalize()
    return nc


_CACHE = {}


def _get_program(nkvs, newpos, hist_ones, tails):
    key = (nkvs, tuple(tuple(int(v) for v in p) for p in newpos), hist_ones,
           tails)
    if key not in _CACHE:
        # descending buffer depths until the SBUF layout fits; drop the
        # shared-tail staging as the last resort before shrinking prefetch
        cfgs = ((6, 4, True), (6, 3, True), (6, 3, False), (5, 3, False),
                (4, 3, False), (4, 2, False), (2, 2, False))
        for i, (wu_bufs, po_bufs, with_tails) in enumerate(cfgs):
            try:
                _CACHE[key] = build_program(
                    nkvs, newpos, wu_bufs=wu_bufs, hist_ones=hist_ones,
                    po_bufs=po_bufs, tails=tails if with_tails else ())
                _CACHE[key]._used_tails = tails if with_tails else ()
                break
            except ValueError:
                if i == len(cfgs) - 1:
                    raise
    return _CACHE[key]


def _prep_inputs(inputs):
    x = np.asarray(inputs["x"], np.float32)
    router = np.asarray(inputs["router"], np.float32)
    cache_k = np.asarray(inputs["cache_k"], np.float32)
    cache_v = np.asarray(inputs["cache_v"], np.float32)
    cache_mask = np.asarray(inputs["cache_mask"])
    mask = np.asarray(inputs["mask"], np.float32)
    wq = np.asarray(inputs["wq"], np.float32)
    wk = np.asarray(inputs["wk"], np.float32)
    wv = np.asarray(inputs["wv"], np.float32)
    wo = np.asarray(inputs["wo"], np.float32)
    position_ids = np.asarray(inputs["position_ids"], np.int64)
    batch_exec = np.asarray(inputs["batch_exec"], np.int64)
    start_pos = int(inputs["start_pos"])
    assert start_pos == START and x.shape == (BSZ, SEQ, DIM)

    # compacted new-token index lists (order-preserving)
    pen_new = router[:, :, 0] != 0.0                                  # [8, 512]
    idx = [np.nonzero(pen_new[b])[0] for b in range(BSZ)]
    # compaction stops paying (and SBUF stops fitting) for dense batches:
    # above 384 usable tokens run uncompacted in original token order, with
    # the router folded into the per-element gate like the history chunks
    idx = [i if len(i) <= 384 else np.arange(SEQ) for i in idx]
    nkvs = tuple(max(16, ((len(i) + 15) // 16) * 16) for i in idx)
    nkv = max(nkvs)
    tcn = (nkv + P - 1) // P
    nch = SC + tcn

    # x packed per batch into the SBUF tile layout [b, p, kc, tok]
    xT = x.reshape(BSZ, SEQ, KC, P)               # tok-major view of features
    xp = np.ascontiguousarray(xT.transpose(0, 3, 2, 1)).astype(BF16)  # [8,128,32,512]
    nkvc = max([n for n in nkvs if n < SEQ], default=16)
    xsel = np.zeros((BSZ, nkvc, KC, P), np.float32)
    for b in range(BSZ):
        if nkvs[b] < SEQ:
            xsel[b, :len(idx[b])] = xT[b, idx[b]]
    xkvp = np.ascontiguousarray(xsel.transpose(0, 3, 2, 1)).astype(BF16)

    # shared V tail chunk: up to 4 batches whose last V chunk would hold
    # <=32 tokens get their tail projected concurrently in b0's phase A
    tails = []
    for b in range(BSZ):
        rem = nkvs[b] % P
        if nkvs[b] < SEQ and nkvs[b] > P and 0 < rem <= 32 and len(tails) < 4:
            tails.append((b, len(tails), rem))
    tails = tuple(tails)
    xtailp = None
    if tails:
        xtail_tok = np.concatenate(
            [xsel[b, nkvs[b] - tl:nkvs[b]] for b, _, tl in tails])  # [nt,KC,P]
        xtailp = np.ascontiguousarray(xtail_tok.transpose(2, 1, 0)).astype(BF16)

    # RoPE tables gathered at position_ids, packed [b, p(hd), table, q|k tok]
    inv_freq = 1.0 / (ROPE_BASE ** (np.arange(0, HD, 2, dtype=np.float32) / HD))
    t = np.arange(KV, dtype=np.float32)
    emb = np.concatenate([t[:, None] * inv_freq, t[:, None] * inv_freq], axis=-1)
    cos_t = np.cos(emb).astype(np.float32)[position_ids]   # [8, 512, 128]
    sin_t = np.sin(emb).astype(np.float32)[position_ids]
    sign = np.where(np.arange(HD) < HD // 2, -1.0, 1.0).astype(np.float32)
    scale = np.float32(1.0 / np.sqrt(HD))
    rope = np.zeros((BSZ, 2, SEQ + nkv, HD), np.float32)
    rope[:, 0, :SEQ] = cos_t * scale
    rope[:, 1, :SEQ] = (sin_t * sign) * scale
    for b in range(BSZ):
        nb = len(idx[b])
        rope[b, 0, SEQ:SEQ + nb] = cos_t[b, idx[b]]
        rope[b, 1, SEQ:SEQ + nb] = sin_t[b, idx[b]] * sign
    ropep = np.ascontiguousarray(rope.transpose(0, 3, 1, 2)).astype(np.float32)

    # history cache slices (host-side gather = sharding)
    k_hist = cache_k[batch_exec, :, :START, :]   # [8, 32, 512, 128]
    v_hist = cache_v[batch_exec, :, :START, :]

    # multiplicative 0/1 gate: causal AND cache-usable, with the new-token
    # half compacted to idx[b]; packed [b, p, chunk, s]
    pen_hist = cache_mask[batch_exec, :START].astype(bool)            # [8, 512]
    causal_ok = (mask[0, 0] > -0.5)                                   # [512 s, 1024 t]
    gate_hist = causal_ok.T[None, :START, :] & pen_hist[:, :, None]   # [8, 512, 512]
    # for this problem's input distribution the history gate is identically
    # 1 (cache_mask all ones, causal mask never blocks history); the program
    # then skips the gate multiply on history chunks entirely
    hist_ones = bool(gate_hist.all())
    gate_new = np.zeros((BSZ, nkv, SEQ), bool)
    for b in range(BSZ):
        nb = len(idx[b])
        gate_new[b, :nb] = causal_ok.T[START + idx[b], :] & pen_new[b, idx[b]][:, None]
    if hist_ones:
        gate = gate_new                                               # [8, nkv, 512]
        gchunks = tcn
    else:
        gate = np.concatenate([gate_hist, gate_new], axis=1)          # [8, 512+nkv, 512]
        gchunks = nch
    pad = gchunks * P - gate.shape[1]
    if pad:
        gate = np.concatenate([gate, np.zeros((BSZ, pad, SEQ), bool)], axis=1)
    gatep = np.ascontiguousarray(
        gate.reshape(BSZ, gchunks, P, SEQ).transpose(0, 2, 1, 3)
        .astype(np.float32)).astype(BF16)                             # [8,128,gch,512]

    in_maps = []
    for c in range(NC):
        hs, he = c * HPC, (c + 1) * HPC
        fs, fe = c * HF, (c + 1) * HF
        # q/k units [2*HPC, p, kc, 128]: unit (proj, h) = W[fs+h*128 : ...].T
        wqkT = np.stack([w[fs:fe].T for w in (wq, wk)])   # [2, 4096, 512]
        wqk = (wqkT.reshape(2, KC, P, HPC, HD).transpose(0, 3, 2, 1, 4)
               .reshape(2 * HPC, P, KC, HD))
        wvT = wv[fs:fe].T                                  # [4096, 512]
        wvp = wvT.reshape(KC, P, HF).transpose(1, 0, 2)    # [128, 32, 512]
        woTc = wo[:, fs:fe].T                              # [512, 4096]
        wop = woTc.reshape(HPC, P, DIM).transpose(1, 0, 2) # [128, 4, 4096]
        kThp = k_hist[:, hs:he].transpose(0, 3, 1, 2)      # [8, 128hd, 4h, 512]
        vhp = (v_hist[:, hs:he].reshape(BSZ, HPC, SC, P, HD)
               .transpose(0, 3, 1, 2, 4).reshape(BSZ, P, HPC, SC * HD))
        attp = np.concatenate(
            [kThp.astype(np.float32), vhp.astype(np.float32),
             gatep.astype(np.float32)], axis=2)            # [8,128,8+gch,512]
        m = {
            "xp": xp,
            "xkvp": xkvp,
            "wqk": np.ascontiguousarray(wqk).astype(BF16),
            "wvp": np.ascontiguousarray(wvp).astype(BF16),
            "wop": np.ascontiguousarray(wop).astype(BF16),
            "ropep": ropep,
            "attp": np.ascontiguousarray(attp).astype(BF16),
        }
        if tails:
            m["xtailp"] = xtailp
        in_maps.append(m)
    return in_maps, nkvs, idx, hist_ones, tails


def _install_profile_hook():
    """The agent image's antenv lacks axon_hooks; shim it so trace=True works."""
    import sys, types
    if "antenv.axon_hooks" in sys.modules:
        return
    try:
        from trn_agent_boot.trn_boot import _ntff_profile_via_ctypes
    except ImportError:
        return
    mod = types.ModuleType("antenv.axon_hooks")
    mod._hook = _ntff_profile_via_ctypes("/opt/axon/libaxon_pjrt.so")

    def set_axon_ntff_profile_hook(h):
        mod._hook = h

    def get_axon_ntff_profile_hook():
        return mod._hook

    mod.set_axon_ntff_profile_hook = set_axon_ntff_profile_hook
    mod.get_axon_ntff_profile_hook = get_axon_ntff_profile_hook
    sys.modules["antenv.axon_hooks"] = mod
    import antenv
    antenv.axon_hooks = mod


def _run(inputs, trace=False):
    if trace:
        _install_profile_hook()
    in_maps, nkvs, newpos, hist_ones, tails = _prep_inputs(inputs)
    nc = _get_program(nkvs, newpos, hist_ones, tails)
    if not getattr(nc, "_used_tails", ()):
        for m in in_maps:
            m.pop("xtailp", None)
    res = run_bass_kernel_spmd(nc, in_maps, core_ids=list(range(NC)), trace=trace)
    acc = np.zeros((BSZ * SEQ, DIM), np.float32)
    for c in range(NC):
        acc += res.results[c]["out"].astype(np.float32)
    return acc.reshape(BSZ, SEQ, DIM), res


def kernel(**inputs):
    out, _ = _run(inputs, trace=False)
    return out



# revision 72
# speedup vs baseline: 1.0308x; 1.0308x over previous
"""Trainium2 Bass kernel: attention layer with KV cache, tensor-parallel over heads.

Sharding (8 NeuronCores): Megatron-style TP over the 32 heads -> 4 heads/core.
  - wq/wk/wv: column-parallel (each core owns a [512, 4096] output shard)
  - wo: row-parallel (each core owns wo[:, c*512:(c+1)*512]); cores emit
    partial o-proj outputs which the host sums (RowParallel unshard).
  - cache_k/cache_v: sharded along the head axis; history rows/positions are
    gathered host-side from batch_exec/start_pos (pure indexing).

Device layout trick: Q/K are projected directly in transposed [head_dim, tok]
layout (head dim = PSUM partitions), V in natural [tok, head_dim] layout, so
scores^T, PV, and the o-projection all consume each other's outputs as
matmul operands with zero on-device transposes.  Softmax runs without the
max-subtraction (scores are O(1), exp is safe in f32) so the kv-axis
(partition-axis) row-sum comes from a ones-vector matmul; normalization is a
rank-1 broadcast matmul of 1/r.  exp runs IN-PLACE in PSUM: the ACT engine's
fast port is PSUM on both sides (the ACT->SBUF write path measured ~8x
slower per element).

New-token compaction: router[:, :, 0] gates ~half the new cache positions
off for every query, so their K/V projections and score/PV columns are dead
work.  The host compacts each batch's unmasked new tokens (order preserved)
and the program projects/attends over pad16(count_b) columns per batch,
with exact causal narrowing from the compacted token positions (chunk at
compact offset j0 only reaches queries s >= orig_pos(j0)).  Batches with
>384 usable tokens fall back to uncompacted identity order (SBUF would not
fit the staging buffer, and compaction stops paying).  The program is built
per-(counts, positions) and cached, so repeated calls with the same router
compile once.

Scheduling: the PE queue is strict FIFO, so emission order is execution
order.  Attention is software-pipelined per head (rowsum/PV matmuls lag two
chunks behind the score matmuls), and the PREVIOUS batch's o-projection
groups are interleaved into the attention beats as dense PE filler, so the
PE never head-of-line blocks on the exp->gate chain.
"""

import numpy as np
import ml_dtypes

import concourse.bass as bass
import concourse.bacc as bacc
import concourse.tile as tile
from concourse import mybir
from concourse.bass_utils import run_bass_kernel_spmd

BF16 = np.dtype(ml_dtypes.bfloat16)

# Problem shape (hardcoded per the task contract)
BSZ = 8
SEQ = 512
DIM = 4096
NH = 32
HD = 128
START = 512
KV = START + SEQ          # 1024
NC = 8                    # cores
HPC = NH // NC            # 4 heads per core
HF = HPC * HD             # 512 local features
P = 128
KC = DIM // P             # 32 contraction chunks
SC = SEQ // P             # 4 seq chunks (also history kv chunks)
ROPE_BASE = 10000.0

FP32 = mybir.dt.float32
BF16D = mybir.dt.bfloat16


def build_program(nkvs, newpos, wu_bufs=4, hist_ones=True, po_bufs=3,
                  tails=()):
    # tails: tuple of (b, slot, tl) — batches whose last V chunk (tl<=32
    # tokens) is projected in batch 0's phase A as one shared 128-partition
    # chunk: each tail's matmuls go to output partitions [32*slot, 32*slot+tl)
    # via col-group tile_position, so the 4 tails' streams run concurrently
    # and 4 nearly-empty chunk passes collapse into one.
    # Every DRAM parameter is pre-packed host-side into the exact SBUF tile
    # layout (partition-major), so each DMA is a contiguous >=8KB-per-partition
    # stream: ~128 fat descriptors instead of thousands of 256B ones.
    # nkvs: per-batch padded compacted new-token counts (multiples of 16).
    # newpos[b][j] = original position of compacted token j (for exact causal
    # narrowing): new chunk starting at compact j0 only reaches queries
    # s >= newpos[b][j0].
    # hist_ones: host verified the history gate (causal AND cache_mask) is
    # all-ones, so history chunks skip the gate multiply (plain copy) and the
    # gate tensor only carries the new-token chunks.
    nkvm = max(nkvs)
    tcnm = (nkvm + P - 1) // P        # max new-token kv chunks
    nchm = SC + tcnm                  # max total kv chunks per head
    # max per-batch OWN v chunks (tail-managed batches keep only full chunks)
    _tbs = {b for b, _, _ in tails}
    vchm = max((nkvs[b] // P) if b in _tbs else (nkvs[b] + P - 1) // P
               for b in range(BSZ))
    gchm = tcnm if hist_ones else nchm  # gate chunks stored
    # batches with nkv == SEQ run uncompacted straight from the resident x
    # tile; the compacted-x staging buffer only needs the compacted batches
    nkvc = max([n for n in nkvs if n < SEQ], default=16)

    tail_of = {b: (slot, tl) for b, slot, tl in tails}
    ntailc = sum(tl for _, _, tl in tails)

    nc = bacc.Bacc(None, target_bir_lowering=False)
    x_d = nc.declare_dram_parameter("xp", [BSZ, P, KC, SEQ], BF16D, isOutput=False)
    xkv_d = nc.declare_dram_parameter("xkvp", [BSZ, P, KC, nkvc], BF16D, isOutput=False)
    if tails:
        xtail_d = nc.declare_dram_parameter("xtailp", [P, KC, ntailc], BF16D, isOutput=False)
    wqk_d = nc.declare_dram_parameter("wqk", [2 * HPC, P, KC, P], BF16D, isOutput=False)
    wv_d = nc.declare_dram_parameter("wvp", [P, KC, HF], BF16D, isOutput=False)
    woT_d = nc.declare_dram_parameter("wop", [P, HPC, DIM], BF16D, isOutput=False)
    rope_d = nc.declare_dram_parameter("ropep", [BSZ, P, 2, SEQ + nkvm], FP32, isOutput=False)
    att_d = nc.declare_dram_parameter("attp", [BSZ, P, 8 + gchm, SEQ], BF16D, isOutput=False)
    out_d = nc.declare_dram_parameter("out", [BSZ * SEQ, DIM], BF16D, isOutput=True)

    from contextlib import ExitStack

    with ExitStack() as ctx:
        tc = ctx.enter_context(tile.TileContext(nc))
        cpool = ctx.enter_context(tc.tile_pool(name="const", bufs=1))
        # 6 bufs = 3 weight units in flight: the K-proj units are consumed in
        # ~3.7us each (272-col matmuls) but take ~4.5us to supply end-to-end,
        # so 2-deep prefetch starves once per batch (and the resulting >3.4us
        # PE idle re-throttles HAM to 1.2GHz for the next ~3.4us).
        wupool = ctx.enter_context(tc.tile_pool(name="wu", bufs=wu_bufs))
        xpool = ctx.enter_context(tc.tile_pool(name="xb", bufs=1))
        rpool = ctx.enter_context(tc.tile_pool(name="rope", bufs=1))
        qkvpool = ctx.enter_context(tc.tile_pool(name="qkv", bufs=2))
        hpool = ctx.enter_context(tc.tile_pool(name="hist", bufs=1))
        # bufs=1: the RoPE mul/mul/mul/add chain is DVE-serial regardless,
        # so double-buffering t1/t2 buys no overlap
        wkpool = ctx.enter_context(tc.tile_pool(name="work", bufs=1))
        epool = ctx.enter_context(tc.tile_pool(name="ee", bufs=2))
        apool = ctx.enter_context(tc.tile_pool(name="at", bufs=2))
        smpool = ctx.enter_context(tc.tile_pool(name="small", bufs=1))
        popool = ctx.enter_context(tc.tile_pool(name="po", bufs=po_bufs))
        pA = ctx.enter_context(tc.tile_pool(name="pA", bufs=2, space="PSUM"))
        pS = ctx.enter_context(tc.tile_pool(name="pS", bufs=2, space="PSUM"))
        pR = ctx.enter_context(tc.tile_pool(name="pR", bufs=1, space="PSUM"))
        pOT = ctx.enter_context(tc.tile_pool(name="pOT", bufs=1, space="PSUM"))
        pP = ctx.enter_context(tc.tile_pool(name="pP", bufs=2, space="PSUM"))
        if True:
            # ---- constants (weights emitted after b0's hot DMAs, see below) ----
            ones_bf = cpool.tile([P, 1], BF16D)
            nc.gpsimd.memset(ones_bf[:], 1.0)
            wv_s = cpool.tile([P, KC, HF], BF16D)
            woT_s = cpool.tile([P, HPC, DIM], BF16D)
            if tails:
                vtailb = cpool.tile([P, HF], BF16D)
                xtailb = cpool.tile([P, KC, ntailc], BF16D)

            def oproj_unit(aT_prev, ts_prev, k, out_eng=None):
                # one (seq-chunk, out-chunk-pair) group of the previous
                # batch's o-projection: dense PE filler between the
                # exp/gate-gated attention beats.
                sc4, og = divmod(k, 4)
                pout = popool.tile([P, 2, HF], BF16D, tag="pout")
                for oi in range(2):
                    oc = og * 2 + oi
                    pp = pP.tile([P, HF], FP32, tag="pp")
                    for h in range(HPC):
                        nc.tensor.matmul(
                            pp[:],
                            aT_prev[:, h, sc4 * P:(sc4 + 1) * P],
                            woT_s[:, h, oc * HF:(oc + 1) * HF],
                            start=(h == 0), stop=(h == HPC - 1),
                        ).annotate("op")
                    nc.vector.tensor_copy(pout[:, oi, :], pp[:]).annotate("opcp")
                (out_eng or nc.gpsimd).dma_start(
                    out_d[ts_prev + sc4 * P: ts_prev + (sc4 + 1) * P,
                          og * 2 * HF:(og * 2 + 2) * HF],
                    pout[:],
                ).annotate("d_out")

            prev_o = None  # (aT, ts) of the batch whose o-proj is pending
            for b in range(BSZ):
                ts = b * SEQ
                nkv = nkvs[b]
                tcn = (nkv + P - 1) // P
                nch = SC + tcn
                vch = (nkv + P - 1) // P
                # b0 runs its phase A as K -> V -> Q: nothing is prefetched
                # yet, and the 4MB x stream alone takes ~11us vs Q-proj's
                # ~7us/head demand. Projecting K (from the small compacted
                # xkv on the scalar queue) and V first lets x land in the
                # background so Q runs dense.
                kfirst = b == 0 and nkv < SEQ
                ufirst = HPC if kfirst else 0
                wuf0 = wupool.tile([P, 16, P], BF16D, tag="wu")
                wuf1 = wupool.tile([P, 16, P], BF16D, tag="wu")
                wu_first = [wuf0, wuf1]
                # at program start the scalar DGE queue is empty: split the
                # first weight unit across both queues so the first matmul
                # group starts ~0.25MB+1MB into the sync stream (b>0 keeps
                # everything on sync -- the scalar queue carries the rope/
                # history/gate tensors there and must not be delayed)
                eng1 = nc.scalar if b == 0 else nc.sync
                for wc in range(4):
                    eng = (nc.sync, nc.sync, eng1, eng1)[wc]
                    eng.dma_start(wu_first[wc // 2][:, (wc % 2) * 8:(wc % 2 + 1) * 8, :],
                                  wqk_d[ufirst, :, wc * 8:(wc + 1) * 8, :]).annotate("d_wu0")
                xb = xpool.tile([P, KC, SEQ], BF16D)
                xkvb = xpool.tile([P, KC, nkvc], BF16D, tag="xkv")

                def emit_x():
                    xsplits = (0, 2, 4, 8, 16, 24, 32) if b == 0 else (0, 8, 16, 24, 32)
                    for xa, xbnd in zip(xsplits, xsplits[1:]):
                        nc.sync.dma_start(
                            xb[:, xa:xbnd, :],
                            x_d[b, :, xa:xbnd, :],
                        ).annotate("d_x")

                def emit_v():
                    srckv = xb if nkv == SEQ else xkvb
                    vch_own = (nkv // P) if b in tail_of else vch
                    for vc in range(vch_own):  # v, natural layout
                        pc = min(P, nkv - vc * P)
                        ps = pA.tile([P, SEQ], FP32, tag="pa")
                        for kc in range(KC):
                            nc.tensor.matmul(
                                ps[0:pc, :HF], srckv[:, kc, vc * P:vc * P + pc], wv_s[:, kc, :],
                                start=(kc == 0), stop=(kc == KC - 1),
                            ).annotate("v")
                        nc.vector.tensor_copy(v_b[0:pc, vc, :], ps[0:pc, :HF]).annotate("vcp")

                if kfirst:
                    # xkv leads the scalar queue: K-proj consumes it ~10us in
                    for xc in range(2):
                        nc.scalar.dma_start(
                            xkvb[:, xc * 16:(xc + 1) * 16, :nkv],
                            xkv_d[b, :, xc * 16:(xc + 1) * 16, :nkv],
                        ).annotate("d_xkv")
                else:
                    emit_x()
                # small hot tensors ride the scalar engine's DGE queues so
                # they are not stuck behind the multi-MB sync-queue streams.
                # They are packed host-side into TWO tensors (rope fp32, rest
                # bf16): each dma_start issue occupies the ACT sequencer for
                # ~3.5us, and the ACT queue also runs the attention exps.
                rope_b = rpool.tile([P, 2, SEQ + nkvm], FP32, tag="rope")
                nc.scalar.dma_start(rope_b[:, :, :SEQ + nkv], rope_d[b, :, :, :SEQ + nkv]).annotate("d_rope")
                gch = tcn if hist_ones else nch
                # attb rows: 0-3 history kT (per head), 4-7 history v (per
                # head, 4x128 tokens), 8+ new-token gate chunks
                attb = hpool.tile([P, 8 + gchm, SEQ], BF16D)
                nc.scalar.dma_start(attb[:, :8 + gch, :], att_d[b, :, :8 + gch, :]).annotate("d_att")

                # ---- phase A: QKV projections (+RoPE for q/k) ----
                qT_b = qkvpool.tile([P, HPC, SEQ], BF16D, tag="qT")
                kT_b = qkvpool.tile([P, HPC, nkvm], BF16D, tag="kT")
                v_b = qkvpool.tile([P, vchm, HF], BF16D, tag="v", bufs=1)

                proj_order = (1, 0) if kfirst else (0, 1)
                for proj in proj_order:  # 0=q (full tokens), 1=k (compacted)
                    if proj == 1 and nkv < SEQ and not kfirst:
                        # xkv is only needed by the k/v units; emitting its
                        # DMA here keeps the q weight-units at the head of
                        # the sync queue (NOTE: the scalar DGE queue is
                        # bandwidth-limited -- bulk streams belong on sync).
                        for xc in range(2):
                            nc.sync.dma_start(
                                xkvb[:, xc * 16:(xc + 1) * 16, :nkv],
                                xkv_d[b, :, xc * 16:(xc + 1) * 16, :nkv],
                            ).annotate("d_xkv")
                    if b == 0 and proj == 1:
                        # wv rides the scalar queue in kfirst mode (the sync
                        # queue carries k-units + x + q-units back-to-back)
                        wv_eng = nc.scalar if kfirst else nc.sync
                        wv_eng.dma_start(wv_s[:, 0:16, :], wv_d[:, 0:16, :]).annotate("d_wv")
                        wv_eng.dma_start(wv_s[:, 16:32, :], wv_d[:, 16:32, :]).annotate("d_wv")
                        if tails:
                            nc.sync.dma_start(xtailb[:], xtail_d[:]).annotate("d_xtail")
                    if kfirst and proj == 0:
                        emit_x()
                    if proj == 0:
                        dst, src, roff, ncols = qT_b, xb, 0, SEQ
                    else:
                        src = xb if nkv == SEQ else xkvb
                        dst, roff, ncols = kT_b, SEQ, nkv
                    for h in range(HPC):
                        if proj == proj_order[0] and h == 0:
                            wu = wu_first
                        else:
                            wu0 = wupool.tile([P, 16, P], BF16D, tag="wu")
                            wu1 = wupool.tile([P, 16, P], BF16D, tag="wu")
                            wu = [wu0, wu1]
                            u = proj * HPC + h
                            # one fat DMA per tile: the 584ns-per-instruction
                            # sync-queue issue cost is what starves the fast
                            # K-proj units, not the transfer itself
                            for half in range(2):
                                nc.sync.dma_start(wu[half][:], wqk_d[u, :, half * 16:half * 16 + 16, :]).annotate("d_wu")
                        ps = pA.tile([P, SEQ], FP32, tag="pa")
                        for kc in range(KC):
                            nc.tensor.matmul(
                                ps[:, :ncols], wu[kc // 16][:, kc % 16, :], src[:, kc, :ncols],
                                start=(kc == 0), stop=(kc == KC - 1),
                            ).annotate("qk")
                        # RoPE: dst = ps*cos + shift64(ps)*sin_signed
                        t1 = wkpool.tile([P, SEQ], FP32, tag="t1")
                        nc.vector.tensor_mul(t1[:, :ncols], ps[:, :ncols], rope_b[:, 0, roff:roff + ncols]).annotate("rope")
                        t2 = wkpool.tile([P, SEQ], FP32, tag="t2")
                        H2 = HD // 2
                        nc.vector.tensor_mul(t2[0:H2, :ncols], ps[H2:P, :ncols], rope_b[0:H2, 1, roff:roff + ncols]).annotate("rope")
                        nc.vector.tensor_mul(t2[H2:P, :ncols], ps[0:H2, :ncols], rope_b[H2:P, 1, roff:roff + ncols]).annotate("rope")
                        nc.vector.tensor_add(dst[:, h, :ncols], t1[:, :ncols], t2[:, :ncols]).annotate("rope")
                    if kfirst and proj == 1:
                        emit_v()

                if not kfirst:
                    emit_v()

                if b == 0 and tails:
                    # shared tail chunk: project every tail batch's leftover
                    # tokens in one pass; the slots' matmuls go to disjoint
                    # 32-aligned output partition groups (distinct col-groups
                    # -> concurrent streams). Each slot accumulates in its OWN
                    # psum bank: interleaved accumulation groups sharing one
                    # bank corrupt each other through the per-bank has_written
                    # clear (measured: slots 1-3 garbage). The attention-score
                    # pool pS is idle during phase A, so its banks serve as
                    # the extra two.
                    psts = []
                    for ti in range(len(tails)):
                        pst = (pA if ti < 2 else pS).tile([P, SEQ], FP32,
                                                          tag=("pa" if ti < 2 else "ps"))
                        psts.append(pst)
                    for kc in range(KC):
                        off = 0
                        for ti, (tb, slot, tl) in enumerate(tails):
                            nc.tensor.matmul(
                                psts[ti][32 * slot:32 * slot + tl, :HF],
                                xtailb[:, kc, off:off + tl], wv_s[:, kc, :],
                                start=(kc == 0), stop=(kc == KC - 1),
                                skip_group_check=True,
                                tile_position=(0, 32 * slot),
                            ).annotate("v")
                            off += tl
                    for ti, (tb, slot, tl) in enumerate(tails):
                        nc.vector.tensor_copy(
                            vtailb[32 * slot:32 * slot + tl, :],
                            psts[ti][32 * slot:32 * slot + tl, :HF]).annotate("vcp")

                if b == 0:
                    # woT is first needed by b0's o-projection, which runs as
                    # filler inside b1's attention (~120us in). Emitting it
                    # here keeps its 4MB out of b0's DMA-deficit window
                    # (b0's phase A already pulls ~12MB against a ~45us span).
                    nc.scalar.dma_start(woT_s[:, 0:2, :], woT_d[:, 0:2, :]).annotate("d_wo")
                    nc.scalar.dma_start(woT_s[:, 2:4, :], woT_d[:, 2:4, :]).annotate("d_wo")

                # ---- phase B: attention, software-pipelined per head ----
                # Chunk list: 4 history chunks (s0=0) then compacted new
                # chunks; new chunk at compact offset j0 only reaches queries
                # s >= j0 (orig position >= compact index), so narrow ops.
                # The PE queue is strict FIFO, so the emission order below IS
                # the execution order: rowsum/PV for chunk ci-2 are emitted
                # between score matmuls so the PE never heads-of-line blocks
                # on the exp->gate chain, and o-proj groups of the PREVIOUS
                # batch are sprinkled in as dense filler.
                aT = apool.tile([P, HPC, SEQ], BF16D)
                ounits = list(range(16)) if prev_o is not None else []
                bslot = tail_of.get(b, (0, 0))[0] * 32 if b in tail_of else 0
                for h in range(HPC):
                    # chunk entries: (k lhsT, v lhsT, pc, s0, p0) where p0 is
                    # the partition base the chunk's scores/ee live at (only
                    # nonzero for the shared-tail chunk)
                    chunks = []
                    for t in range(SC):
                        chunks.append((attb[:, h, t * P:(t + 1) * P],
                                       attb[:, HPC + h, t * P:(t + 1) * P], P, 0, 0))
                    for tcn_i in range(tcn):
                        j0 = tcn_i * P
                        pc = min(P, nkv - j0)
                        s0 = int(newpos[b][j0]) if j0 < len(newpos[b]) else SEQ - 1
                        # round the causal-narrowing start down to 8 elements:
                        # odd s0 makes every narrowed PSUM/SBUF access pattern
                        # straddle the 8-byte PSUM cachelines; the gate is
                        # genuinely zero on [s0&~7, s0) so the result is
                        # unchanged
                        s0 &= ~7
                        if b in tail_of and tcn_i == tcn - 1:
                            chunks.append((kT_b[:, h, j0:j0 + pc],
                                           vtailb[bslot:bslot + pc, h * HD:(h + 1) * HD],
                                           pc, s0, bslot))
                        else:
                            chunks.append((kT_b[:, h, j0:j0 + pc],
                                           v_b[0:pc, tcn_i, h * P:(h + 1) * P], pc, s0, 0))
                    # ee is a 4-slot ring over kv chunks: rs/pv consume chunk
                    # ci two chunks behind the score matmuls, so slot ci%4 is
                    # free again by the time chunk ci+4 writes it
                    ee = epool.tile([P, 4, SEQ], BF16D)
                    pr = pR.tile([1, SEQ], FP32, tag="pr")
                    po = pOT.tile([P, SEQ], FP32, tag="po")

                    def rs_pv(ci):
                        _, vlhs, pc, s0, p0 = chunks[ci]
                        nc.tensor.matmul(
                            pr[:, s0:], ones_bf[p0:p0 + pc, :], ee[p0:p0 + pc, ci % 4, s0:],
                            start=(ci == 0), stop=(ci == nch - 1),
                            skip_group_check=True,
                            tile_position=(p0, 0),
                        ).annotate("rs")
                        nc.tensor.matmul(
                            po[:, s0:], vlhs, ee[p0:p0 + pc, ci % 4, s0:],
                            start=(ci == 0), stop=(ci == nch - 1),
                            skip_group_check=True,
                            tile_position=(p0, 0),
                        ).annotate("pv")

                    for ci, (klhs, vlhs, pc, s0, p0) in enumerate(chunks):
                        pscr = pS.tile([P, SEQ], FP32, tag="ps")
                        nc.tensor.matmul(pscr[p0:p0 + pc, s0:], klhs, qT_b[:, h, s0:],
                                         start=True, stop=True,
                                         tile_position=(0, p0)).annotate("sc")
                        # exp in-place in PSUM: ScalarE's fast port is PSUM on
                        # both sides; ACT->SBUF measured ~8x slower.
                        nc.scalar.activation(pscr[p0:p0 + pc, s0:], pscr[p0:p0 + pc, s0:], mybir.ActivationFunctionType.Exp).annotate("exp")
                        if hist_ones and ci < SC:
                            # history gate is provably all-ones: plain copy
                            nc.vector.tensor_copy(ee[0:pc, ci % 4, s0:], pscr[0:pc, s0:]).annotate("gate")
                        else:
                            gci = ci - SC if hist_ones else ci
                            nc.vector.tensor_mul(ee[p0:p0 + pc, ci % 4, s0:], pscr[p0:p0 + pc, s0:], attb[0:pc, 8 + gci, s0:]).annotate("gate")
                        if ci >= 2:
                            rs_pv(ci - 2)
                            if ci % 2 == 0 and ounits:
                                oproj_unit(prev_o[0], prev_o[1], ounits.pop(0))
                    rs_pv(nch - 2)
                    if ounits:
                        oproj_unit(prev_o[0], prev_o[1], ounits.pop(0))
                    rs_pv(nch - 1)
                    rinv = smpool.tile([1, SEQ], FP32, tag="rinv")
                    nc.vector.reciprocal_approx_fast(rinv[:], pr[:])
                    rb_s = smpool.tile([P, SEQ], FP32, tag="rbs")
                    nc.gpsimd.partition_broadcast(rb_s[:], rinv[:]).annotate("rbc")
                    nc.vector.tensor_mul(aT[:, h, :], po[:], rb_s[:]).annotate("norm")
                # drain any o-proj groups this batch's slots didn't absorb
                while ounits:
                    oproj_unit(prev_o[0], prev_o[1], ounits.pop(0))
                prev_o = (aT, ts)

            # final batch's o-projection (no next attention phase to hide in)
            for k in range(16):
                oproj_unit(prev_o[0], prev_o[1], k)
    nc.finalize()
    return nc


_CACHE = {}


def _get_program(nkvs, newpos, hist_ones, tails):
    key = (nkvs, tuple(tuple(int(v) for v in p) for p in newpos), hist_ones,
           tails)
    if key not in _CACHE:
        # descending buffer depths until the SBUF layout fits; drop the
        # shared-tail staging as the last resort before shrinking prefetch
        cfgs = ((6, 4, True), (6, 3, True), (6, 3, False), (5, 3, False),
                (4, 3, False), (4, 2, False), (2, 2, False))
        for i, (wu_bufs, po_bufs, with_tails) in enumerate(cfgs):
            try:
                _CACHE[key] = build_program(
                    nkvs, newpos, wu_bufs=wu_bufs, hist_ones=hist_ones,
                    po_bufs=po_bufs, tails=tails if with_tails else ())
                _CACHE[key]._used_tails = tails if with_tails else ()
                break
            except ValueError:
                if i == len(cfgs) - 1:
                    raise
    return _CACHE[key]


def _prep_inputs(inputs):
    x = np.asarray(inputs["x"], np.float32)
    router = np.asarray(inputs["router"], np.float32)
    cache_k = np.asarray(inputs["cache_k"], np.float32)
    cache_v = np.asarray(inputs["cache_v"], np.float32)
    cache_mask = np.asarray(inputs["cache_mask"])
    mask = np.asarray(inputs["mask"], np.float32)
    wq = np.asarray(inputs["wq"], np.float32)
    wk = np.asarray(inputs["wk"], np.float32)
    wv = np.asarray(inputs["wv"], np.float32)
    wo = np.asarray(inputs["wo"], np.float32)
    position_ids = np.asarray(inputs["position_ids"], np.int64)
    batch_exec = np.asarray(inputs["batch_exec"], np.int64)
    start_pos = int(inputs["start_pos"])
    assert start_pos == START and x.shape == (BSZ, SEQ, DIM)

    # compacted new-token index lists (order-preserving)
    pen_new = router[:, :, 0] != 0.0                                  # [8, 512]
    idx = [np.nonzero(pen_new[b])[0] for b in range(BSZ)]
    # compaction stops paying (and SBUF stops fitting) for dense batches:
    # above 384 usable tokens run uncompacted in original token order, with
    # the router folded into the per-element gate like the history chunks
    idx = [i if len(i) <= 384 else np.arange(SEQ) for i in idx]
    nkvs = tuple(max(16, ((len(i) + 15) // 16) * 16) for i in idx)
    nkv = max(nkvs)
    tcn = (nkv + P - 1) // P
    nch = SC + tcn

    # x packed per batch into the SBUF tile layout [b, p, kc, tok]
    xT = x.reshape(BSZ, SEQ, KC, P)               # tok-major view of features
    xp = np.ascontiguousarray(xT.transpose(0, 3, 2, 1)).astype(BF16)  # [8,128,32,512]
    nkvc = max([n for n in nkvs if n < SEQ], default=16)
    xsel = np.zeros((BSZ, nkvc, KC, P), np.float32)
    for b in range(BSZ):
        if nkvs[b] < SEQ:
            xsel[b, :len(idx[b])] = xT[b, idx[b]]
    xkvp = np.ascontiguousarray(xsel.transpose(0, 3, 2, 1)).astype(BF16)

    # shared V tail chunk: up to 4 batches whose last V chunk would hold
    # <=32 tokens get their tail projected concurrently in b0's phase A
    tails = []
    for b in range(BSZ):
        rem = nkvs[b] % P
        if nkvs[b] < SEQ and nkvs[b] > P and 0 < rem <= 32 and len(tails) < 4:
            tails.append((b, len(tails), rem))
    tails = tuple(tails)
    xtailp = None
    if tails:
        xtail_tok = np.concatenate(
            [xsel[b, nkvs[b] - tl:nkvs[b]] for b, _, tl in tails])  # [nt,KC,P]
        xtailp = np.ascontiguousarray(xtail_tok.transpose(2, 1, 0)).astype(BF16)

    # RoPE tables gathered at position_ids, packed [b, p(hd), table, q|k tok]
    inv_freq = 1.0 / (ROPE_BASE ** (np.arange(0, HD, 2, dtype=np.float32) / HD))
    t = np.arange(KV, dtype=np.float32)
    emb = np.concatenate([t[:, None] * inv_freq, t[:, None] * inv_freq], axis=-1)
    cos_t = np.cos(emb).astype(np.float32)[position_ids]   # [8, 512, 128]
    sin_t = np.sin(emb).astype(np.float32)[position_ids]
    sign = np.where(np.arange(HD) < HD // 2, -1.0, 1.0).astype(np.float32)
    scale = np.float32(1.0 / np.sqrt(HD))
    rope = np.zeros((BSZ, 2, SEQ + nkv, HD), np.float32)
    rope[:, 0, :SEQ] = cos_t * scale
    rope[:, 1, :SEQ] = (sin_t * sign) * scale
    for b in range(BSZ):
        nb = len(idx[b])
        rope[b, 0, SEQ:SEQ + nb] = cos_t[b, idx[b]]
        rope[b, 1, SEQ:SEQ + nb] = sin_t[b, idx[b]] * sign
    ropep = np.ascontiguousarray(rope.transpose(0, 3, 1, 2)).astype(np.float32)

    # history cache slices (host-side gather = sharding)
    k_hist = cache_k[batch_exec, :, :START, :]   # [8, 32, 512, 128]
    v_hist = cache_v[batch_exec, :, :START, :]

    # multiplicative 0/1 gate: causal AND cache-usable, with the new-token
    # half compacted to idx[b]; packed [b, p, chunk, s]
    pen_hist = cache_mask[batch_exec, :START].astype(bool)            # [8, 512]
    causal_ok = (mask[0, 0] > -0.5)                                   # [512 s, 1024 t]
    gate_hist = causal_ok.T[None, :START, :] & pen_hist[:, :, None]   # [8, 512, 512]
    # for this problem's input distribution the history gate is identically
    # 1 (cache_mask all ones, causal mask never blocks history); the program
    # then skips the gate multiply on history chunks entirely
    hist_ones = bool(gate_hist.all())
    gate_new = np.zeros((BSZ, nkv, SEQ), bool)
    for b in range(BSZ):
        nb = len(idx[b])
        gate_new[b, :nb] = causal_ok.T[START + idx[b], :] & pen_new[b, idx[b]][:, None]
    if hist_ones:
        gate = gate_new                                               # [8, nkv, 512]
        gchunks = tcn
    else:
        gate = np.concatenate([gate_hist, gate_new], axis=1)          # [8, 512+nkv, 512]
        gchunks = nch
    pad = gchunks * P - gate.shape[1]
    if pad:
        gate = np.concatenate([gate, np.zeros((BSZ, pad, SEQ), bool)], axis=1)
    gatep = np.ascontiguousarray(
        gate.reshape(BSZ, gchunks, P, SEQ).transpose(0, 2, 1, 3)
        .astype(np.float32)).astype(BF16)                             # [8,128,gch,512]

    in_maps = []
    for c in range(NC):
        hs, he = c * HPC, (c + 1) * HPC
        fs, fe = c * HF, (c + 1) * HF
        # q/k units [2*HPC, p, kc, 128]: unit (proj, h) = W[fs+h*128 : ...].T
        wqkT = np.stack([w[fs:fe].T for w in (wq, wk)])   # [2, 4096, 512]
        wqk = (wqkT.reshape(2, KC, P, HPC, HD).transpose(0, 3, 2, 1, 4)
               .reshape(2 * HPC, P, KC, HD))
        wvT = wv[fs:fe].T                                  # [4096, 512]
        wvp = wvT.reshape(KC, P, HF).transpose(1, 0, 2)    # [128, 32, 512]
        woTc = wo[:, fs:fe].T                              # [512, 4096]
        wop = woTc.reshape(HPC, P, DIM).transpose(1, 0, 2) # [128, 4, 4096]
        kThp = k_hist[:, hs:he].transpose(0, 3, 1, 2)      # [8, 128hd, 4h, 512]
        vhp = (v_hist[:, hs:he].reshape(BSZ, HPC, SC, P, HD)
               .transpose(0, 3, 1, 2, 4).reshape(BSZ, P, HPC, SC * HD))
        attp = np.concatenate(
            [kThp.astype(np.float32), vhp.astype(np.float32),
             gatep.astype(np.float32)], axis=2)            # [8,128,8+gch,512]
        m = {
            "xp": xp,
            "xkvp": xkvp,
            "wqk": np.ascontiguousarray(wqk).astype(BF16),
            "wvp": np.ascontiguousarray(wvp).astype(BF16),
            "wop": np.ascontiguousarray(wop).astype(BF16),
            "ropep": ropep,
            "attp": np.ascontiguousarray(attp).astype(BF16),
        }
        if tails:
            m["xtailp"] = xtailp
        in_maps.append(m)
    return in_maps, nkvs, idx, hist_ones, tails


def _install_profile_hook():
    """The agent image's antenv lacks axon_hooks; shim it so trace=True works."""
    import sys, types
    if "antenv.axon_hooks" in sys.modules:
        return
    try:
        from trn_agent_boot.trn_boot import _ntff_profile_via_ctypes
    except ImportError:
        return
    mod = types.ModuleType("antenv.axon_hooks")
    mod._hook = _ntff_profile_via_ctypes("/opt/axon/libaxon_pjrt.so")

    def set_axon_ntff_profile_hook(h):
        mod._hook = h

    def get_axon_ntff_profile_hook():
        return mod._hook

    mod.set_axon_ntff_profile_hook = set_axon_ntff_profile_hook
    mod.get_axon_ntff_profile_hook = get_axon_ntff_profile_hook
    sys.modules["antenv.axon_hooks"] = mod
    import antenv
    antenv.axon_hooks = mod


def _run(inputs, trace=False):
    if trace:
        _install_profile_hook()
    in_maps, nkvs, newpos, hist_ones, tails = _prep_inputs(inputs)
    nc = _get_program(nkvs, newpos, hist_ones, tails)
    if not getattr(nc, "_used_tails", ()):
        for m in in_maps:
            m.pop("xtailp", None)
    res = run_bass_kernel_spmd(nc, in_maps, core_ids=list(range(NC)), trace=trace)
    acc = np.zeros((BSZ * SEQ, DIM), np.float32)
    for c in range(NC):
        acc += res.results[c]["out"].astype(np.float32)
    return acc.reshape(BSZ, SEQ, DIM), res


def kernel(**inputs):
    out, _ = _run(inputs, trace=False)
    return out

